# revision 11
# baseline (speedup 1.0000x reference)
"""Trainium2 Bass kernel for nn_Attention_73813307949260.

Reference computation (per example b of 80, L=256, S=512, H=1024):
    ex_attn  = softmax(output @ context0^T)            [L, L]
    set_attn = softmax(output @ set_ctx^T)             [L, S]   set_ctx = context1[b // 10]
    ex_c     = ex_attn @ context0                      [L, H]
    set_c    = set_attn @ set_ctx                      [L, H]
    out      = tanh(concat([output, ex_c, set_c]) @ W_out^T + b_out)
    returns (out, ex_attn, set_attn)                   (mask is unused upstream)

Sharding: data-parallel over the 80-example batch axis, 10 examples per core;
core i's 10 examples all share set_ctx = context1[i]. W_out/b_out replicated.

Kernel algebra (avoids materializing ex_c / set_c):
    out = tanh(output @ W0^T + ex_attn @ M1 + set_attn @ (M2 + 1*b))
    M1 = context0 @ W1^T (per example), M2 = set_ctx @ W2^T (once per core),
    where W_out = [W0 | W1 | W2] split along its 3H input axis. The bias rides
    on M2 because softmax rows sum to 1.

Precision: all matmuls in fp16 (10-bit mantissa) with fp32 PSUM accumulation.
exp() amplifies the *absolute* score error, so bf16 scores (abs err ~0.3 for
std-32 scores) would corrupt the attention weights; fp16 keeps it at ~0.03.
Softmax itself runs in fp32 on the Scalar/Vector engines.
"""

import numpy as np

import concourse.bacc as bacc
import concourse.mybir as mybir
import concourse.tile as tile
from concourse.bass_utils import run_bass_kernel_spmd
from concourse.masks import make_identity

N_CORES = 8
NE = 10          # examples per core
L = 256          # query length
S = 512          # set context length
H = 1024         # hidden
KT = H // 128    # contraction tiles for H
F32 = mybir.dt.float32
FP16 = mybir.dt.float16
AX = mybir.AxisListType.X
ACT = mybir.ActivationFunctionType

_CACHE = {}


def _build():
    nc = bacc.Bacc("TRN2", num_devices=N_CORES)

    # Host passes contraction-major ("transposed") fp16 tensors so every
    # matmul operand loads with the contraction dim on partitions.
    qt = nc.declare_dram_parameter("qt", [NE, H, L], FP16, isOutput=False)
    k0t = nc.declare_dram_parameter("k0t", [NE, H, L], FP16, isOutput=False)
    c1t = nc.declare_dram_parameter("c1t", [H, S], FP16, isOutput=False)
    w01 = nc.declare_dram_parameter("w01", [2 * H, H], FP16, isOutput=False)
    w2 = nc.declare_dram_parameter("w2", [H, H], FP16, isOutput=False)
    bias = nc.declare_dram_parameter("bias", [1, H], FP16, isOutput=False)
    out = nc.declare_dram_parameter("out", [NE, L, H], F32, isOutput=True)
    exattn = nc.declare_dram_parameter("exattn", [NE, L, L], F32, isOutput=True)
    setattn = nc.declare_dram_parameter("setattn", [NE, L, S], F32, isOutput=True)

    def dma_tiled(dst, src, n_kt, chunks):
        """DMA a [n_kt*128, w] DRAM tensor (or [n_kt, 128, w] view) into a
        [128, n_kt, w] tile, split into `chunks` DMAs across HWDGE queues."""
        per = n_kt // chunks
        if len(src.shape) == 2:
            view = src.rearrange("(kt p) w -> p kt w", p=128)
        else:
            view = src.rearrange("kt p w -> p kt w")
        for c in range(chunks):
            nc.sync.dma_start(
                dst[:, c * per:(c + 1) * per, :],
                view[:, c * per:(c + 1) * per, :],
            )

    with tile.TileContext(nc) as tc:
        with (
            tc.tile_pool(name="const", bufs=1) as const_pool,
            tc.tile_pool(name="qk", bufs=3) as qk_pool,
            tc.tile_pool(name="m1", bufs=2) as m1_pool,
            tc.tile_pool(name="att", bufs=2) as att_pool,
            tc.tile_pool(name="soft", bufs=6) as soft_pool,
            tc.tile_pool(name="stats", bufs=12) as stats_pool,
            tc.tile_pool(name="osb", bufs=3) as osb_pool,
            tc.tile_pool(name="ps_main", bufs=4, space="PSUM") as ps_main,
            tc.tile_pool(name="ps_sc", bufs=2, space="PSUM") as ps_sc,
            tc.tile_pool(name="ps_tr", bufs=2, space="PSUM") as ps_tr,
        ):
            # ---- example input loads (ex 0/1 first so the PE starts ASAP) ----
            ex_in = {}

            def load_example(i, chunks):
                qt_i = qk_pool.tile([128, KT, L], FP16, tag="qt")
                k0t_i = qk_pool.tile([128, KT, L], FP16, tag="k0t")
                dma_tiled(qt_i, qt[i], KT, chunks)
                dma_tiled(k0t_i, k0t[i], KT, chunks)
                ex_in[i] = (qt_i, k0t_i)

            bias_t = const_pool.tile([1, H], FP16)
            nc.sync.dma_start(bias_t[:], bias[:])
            load_example(0, 8)
            c1t_t = const_pool.tile([128, KT, S], FP16)
            dma_tiled(c1t_t, c1t, KT, 4)
            load_example(1, 4)
            w2_t = const_pool.tile([128, KT, H], FP16)
            dma_tiled(w2_t, w2, KT, 2)
            w01_t = const_pool.tile([128, 2 * KT, H], FP16)
            w01v = w01.rearrange("(a k) h -> a k h", a=2)
            dma_tiled(w01_t[:, KT:2 * KT, :], w01v[1], KT, 2)  # W1 needed first
            dma_tiled(w01_t[:, 0:KT, :], w01v[0], KT, 2)

            ident_h = const_pool.tile([128, 128], FP16)
            make_identity(nc, ident_h[:])
            ones_t = const_pool.tile([1, 128], FP16)
            nc.gpsimd.memset(ones_t[:], 1.0)

            m2_t = const_pool.tile([128, S // 128, H], FP16)

            def emit_m2():
                # M2 = set_ctx @ W2^T + 1*b  [S, H] fp16, once per core
                for si in range(S // 128):
                    for hf in range(2):
                        hsl = slice(hf * 512, (hf + 1) * 512)
                        ps = ps_main.tile([128, 512], F32, tag="ps")
                        for ki in range(KT):
                            nc.tensor.matmul(
                                ps[:],
                                c1t_t[:, ki, si * 128:(si + 1) * 128],
                                w2_t[:, ki, hsl],
                                start=(ki == 0),
                                stop=False,
                            )
                        nc.tensor.matmul(
                            ps[:], ones_t[0:1, :], bias_t[0:1, hsl],
                            start=False, stop=True,
                        )
                        nc.scalar.copy(m2_t[:, si, hsl], ps[:])

            # ---- per-example pipeline: scores run 2 examples ahead ----
            def emit_scores(i):
                if i not in ex_in:
                    load_example(i, 2)
                qt_i, k0t_i = ex_in.pop(i)
                attn_ex_tiles = []
                attn_set_tiles = []
                for li in range(L // 128):
                    lsl = slice(li * 128, (li + 1) * 128)
                    ps_ex = ps_sc.tile([128, 512], F32, tag="psc")
                    for ki in range(KT):
                        nc.tensor.matmul(
                            ps_ex[:, 0:L],
                            qt_i[:, ki, lsl],
                            k0t_i[:, ki, :],
                            start=(ki == 0),
                            stop=(ki == KT - 1),
                        )
                    ps_set = ps_sc.tile([128, 512], F32, tag="psc")
                    for ki in range(KT):
                        nc.tensor.matmul(
                            ps_set[:],
                            qt_i[:, ki, lsl],
                            c1t_t[:, ki, :],
                            start=(ki == 0),
                            stop=(ki == KT - 1),
                        )

                    # softmax over the free axis: p = exp(s - max), r = 1/sum
                    attn_ex = soft_pool.tile([128, L], F32, tag="attn_ex")
                    attn_set = soft_pool.tile([128, S], F32, tag="attn_set")
                    attn_ex_h = soft_pool.tile([128, L], FP16, tag="attn_ex_h")
                    attn_set_h = soft_pool.tile([128, S], FP16, tag="attn_set_h")
                    for ps_s, attn, attn_h in (
                        (ps_ex[:, 0:L], attn_ex, attn_ex_h),
                        (ps_set[:], attn_set, attn_set_h),
                    ):
                        nmx = stats_pool.tile([128, 1], F32, tag="nmx")
                        nc.vector.reduce_max(nmx[:], ps_s, axis=AX, negate=True)
                        psum_r = stats_pool.tile([128, 1], F32, tag="psum")
                        nc.scalar.activation(
                            attn[:], ps_s, ACT.Exp, bias=nmx[:], accum_out=psum_r[:]
                        )
                        rinv = stats_pool.tile([128, 1], F32, tag="rinv")
                        nc.vector.reciprocal(rinv[:], psum_r[:])
                        nc.vector.tensor_scalar_mul(attn_h[:], attn[:], rinv[:])
                        nc.vector.tensor_scalar_mul(attn[:], attn[:], rinv[:])

                    nc.sync.dma_start(exattn[i, lsl, :], attn_ex[:])
                    nc.sync.dma_start(setattn[i, lsl, :], attn_set[:])
                    attn_ex_tiles.append(attn_ex_h)
                    attn_set_tiles.append(attn_set_h)
                return qt_i, k0t_i, attn_ex_tiles, attn_set_tiles

            def emit_rest(i, staged):
                qt_i, k0t_i, attn_ex_tiles, attn_set_tiles = staged

                # M1 = context0 @ W1^T  [L, H] fp16
                m1_t = m1_pool.tile([128, L // 128, H], FP16, tag="m1")
                for si in range(L // 128):
                    for hf in range(2):
                        ps = ps_main.tile([128, 512], F32, tag="ps")
                        for ki in range(KT):
                            nc.tensor.matmul(
                                ps[:],
                                k0t_i[:, ki, si * 128:(si + 1) * 128],
                                w01_t[:, KT + ki, hf * 512:(hf + 1) * 512],
                                start=(ki == 0),
                                stop=(ki == KT - 1),
                            )
                        nc.scalar.copy(m1_t[:, si, hf * 512:(hf + 1) * 512], ps[:])

                # transpose attn -> [s_part, s_tile, l] fp16 (PE transpose)
                at_ex = att_pool.tile([128, L // 128, L], FP16, tag="at_ex")
                for si in range(L // 128):
                    ps_t = ps_tr.tile([128, L], FP16, tag="tr")
                    for li in range(L // 128):
                        nc.tensor.transpose(
                            ps_t[:, li * 128:(li + 1) * 128],
                            attn_ex_tiles[li][:, si * 128:(si + 1) * 128],
                            ident_h[:],
                        )
                    nc.vector.tensor_copy(at_ex[:, si, :], ps_t[:])
                at_set = att_pool.tile([128, S // 128, L], FP16, tag="at_set")
                for si in range(S // 128):
                    ps_t = ps_tr.tile([128, L], FP16, tag="tr")
                    for li in range(L // 128):
                        nc.tensor.transpose(
                            ps_t[:, li * 128:(li + 1) * 128],
                            attn_set_tiles[li][:, si * 128:(si + 1) * 128],
                            ident_h[:],
                        )
                    nc.vector.tensor_copy(at_set[:, si, :], ps_t[:])

                # out = tanh(q @ W0^T + ex_attn @ M1 + set_attn @ (M2 + 1*b))
                for li in range(L // 128):
                    lsl = slice(li * 128, (li + 1) * 128)
                    for hf in range(2):
                        hsl = slice(hf * 512, (hf + 1) * 512)
                        ps = ps_main.tile([128, 512], F32, tag="ps")
                        for ki in range(KT):
                            nc.tensor.matmul(
                                ps[:], qt_i[:, ki, lsl], w01_t[:, ki, hsl],
                                start=(ki == 0), stop=False,
                            )
                        for si in range(L // 128):
                            nc.tensor.matmul(
                                ps[:], at_ex[:, si, lsl], m1_t[:, si, hsl],
                                start=False, stop=False,
                            )
                        for si in range(S // 128):
                            nc.tensor.matmul(
                                ps[:], at_set[:, si, lsl], m2_t[:, si, hsl],
                                start=False, stop=(si == S // 128 - 1),
                            )
                        osb = osb_pool.tile([128, 512], F32, tag="osb")
                        nc.scalar.activation(osb[:], ps[:], ACT.Tanh)
                        nc.sync.dma_start(out[i, lsl, hsl], osb[:])

            staged = {}
            staged[0] = emit_scores(0)
            staged[1] = emit_scores(1)
            emit_m2()
            for i in range(NE):
                if i + 2 < NE:
                    staged[i + 2] = emit_scores(i + 2)
                emit_rest(i, staged.pop(i))

    nc.compile()
    return nc


def _get_nc():
    if "nc" not in _CACHE:
        _CACHE["nc"] = _build()
    return _CACHE["nc"]


def kernel_run(output, context0, context1, mask, W_out, b_out, **run_kwargs):
    del mask  # reference discards the masked_fill result; mask is a no-op
    nc = _get_nc()

    # contraction-major fp16 host layouts
    qt = np.ascontiguousarray(output.transpose(0, 2, 1)).astype(np.float16)
    k0t = np.ascontiguousarray(context0.transpose(0, 2, 1)).astype(np.float16)
    c1t = np.ascontiguousarray(context1.transpose(0, 2, 1)).astype(np.float16)
    wt = W_out.T.astype(np.float16)  # [3H, H]
    w01 = np.ascontiguousarray(wt[:2 * H])
    w2 = np.ascontiguousarray(wt[2 * H:])
    b = np.asarray(b_out, dtype=np.float16).reshape(1, H)

    in_maps = []
    for c in range(N_CORES):
        in_maps.append({
            "qt": qt[c * NE:(c + 1) * NE],
            "k0t": k0t[c * NE:(c + 1) * NE],
            "c1t": c1t[c],
            "w01": w01,
            "w2": w2,
            "bias": b,
        })

    res = run_bass_kernel_spmd(nc, in_maps, core_ids=list(range(N_CORES)), **run_kwargs)
    out = np.concatenate([r["out"] for r in res.results], axis=0)
    ex_attn = np.concatenate([r["exattn"] for r in res.results], axis=0)
    set_attn = np.concatenate([r["setattn"] for r in res.results], axis=0)
    return (out, ex_attn, set_attn), res


def kernel(output, context0, context1, mask, W_out, b_out):
    (out, ex_attn, set_attn), _ = kernel_run(
        output, context0, context1, mask, W_out, b_out
    )
    return out, ex_attn, set_attn


# revision 12
# speedup vs baseline: 1.0266x; 1.0266x over previous
"""Trainium2 Bass kernel for nn_Attention_73813307949260.

Reference computation (per example b of 80, L=256, S=512, H=1024):
    ex_attn  = softmax(output @ context0^T)            [L, L]
    set_attn = softmax(output @ set_ctx^T)             [L, S]   set_ctx = context1[b // 10]
    ex_c     = ex_attn @ context0                      [L, H]
    set_c    = set_attn @ set_ctx                      [L, H]
    out      = tanh(concat([output, ex_c, set_c]) @ W_out^T + b_out)
    returns (out, ex_attn, set_attn)                   (mask is unused upstream)

Sharding: data-parallel over the 80-example batch axis, 10 examples per core;
core i's 10 examples all share set_ctx = context1[i]. W_out/b_out replicated.

Kernel algebra (avoids materializing ex_c / set_c):
    out = tanh(output @ W0^T + ex_attn @ M1 + set_attn @ (M2 + 1*b))
    M1 = context0 @ W1^T (per example), M2 = set_ctx @ W2^T (once per core),
    where W_out = [W0 | W1 | W2] split along its 3H input axis. The bias rides
    on M2 because softmax rows sum to 1.

Precision: all matmuls in fp16 (10-bit mantissa) with fp32 PSUM accumulation.
exp() amplifies the *absolute* score error, so bf16 scores (abs err ~0.3 for
std-32 scores) would corrupt the attention weights; fp16 keeps it at ~0.03.
Softmax itself runs in fp32 on the Scalar/Vector engines.
"""

import numpy as np

import concourse.bacc as bacc
import concourse.mybir as mybir
import concourse.tile as tile
from concourse.bass_utils import run_bass_kernel_spmd
from concourse.masks import make_identity

N_CORES = 8
NE = 10          # examples per core
L = 256          # query length
S = 512          # set context length
H = 1024         # hidden
KT = H // 128    # contraction tiles for H
F32 = mybir.dt.float32
FP16 = mybir.dt.float16
AX = mybir.AxisListType.X
ACT = mybir.ActivationFunctionType

_CACHE = {}


def _build():
    nc = bacc.Bacc("TRN2", num_devices=N_CORES)

    # Host passes contraction-major ("transposed") fp16 tensors so every
    # matmul operand loads with the contraction dim on partitions.
    qt = nc.declare_dram_parameter("qt", [NE, H, L], FP16, isOutput=False)
    k0t = nc.declare_dram_parameter("k0t", [NE, H, L], FP16, isOutput=False)
    c1t = nc.declare_dram_parameter("c1t", [H, S], FP16, isOutput=False)
    w01 = nc.declare_dram_parameter("w01", [2 * H, H], FP16, isOutput=False)
    w2 = nc.declare_dram_parameter("w2", [H, H], FP16, isOutput=False)
    bias = nc.declare_dram_parameter("bias", [1, H], FP16, isOutput=False)
    out = nc.declare_dram_parameter("out", [NE, L, H], F32, isOutput=True)
    exattn = nc.declare_dram_parameter("exattn", [NE, L, L], F32, isOutput=True)
    setattn = nc.declare_dram_parameter("setattn", [NE, L, S], F32, isOutput=True)

    def dma_tiled(dst, src, n_kt, chunks):
        """DMA a [n_kt*128, w] DRAM tensor (or [n_kt, 128, w] view) into a
        [128, n_kt, w] tile, split into `chunks` DMAs across HWDGE queues."""
        per = n_kt // chunks
        if len(src.shape) == 2:
            view = src.rearrange("(kt p) w -> p kt w", p=128)
        else:
            view = src.rearrange("kt p w -> p kt w")
        for c in range(chunks):
            nc.sync.dma_start(
                dst[:, c * per:(c + 1) * per, :],
                view[:, c * per:(c + 1) * per, :],
            )

    with tile.TileContext(nc) as tc:
        with (
            tc.tile_pool(name="const", bufs=1) as const_pool,
            tc.tile_pool(name="qk", bufs=3) as qk_pool,
            tc.tile_pool(name="m1", bufs=2) as m1_pool,
            tc.tile_pool(name="att", bufs=2) as att_pool,
            tc.tile_pool(name="soft", bufs=6) as soft_pool,
            tc.tile_pool(name="stats", bufs=12) as stats_pool,
            tc.tile_pool(name="osb", bufs=3) as osb_pool,
            tc.tile_pool(name="ps_main", bufs=4, space="PSUM") as ps_main,
            tc.tile_pool(name="ps_sc", bufs=2, space="PSUM") as ps_sc,
            tc.tile_pool(name="ps_tr", bufs=2, space="PSUM") as ps_tr,
        ):
            # ---- example input loads (ex 0/1 first so the PE starts ASAP) ----
            ex_in = {}

            def load_example(i, chunks):
                qt_i = qk_pool.tile([128, KT, L], FP16, tag="qt")
                k0t_i = qk_pool.tile([128, KT, L], FP16, tag="k0t")
                dma_tiled(qt_i, qt[i], KT, chunks)
                dma_tiled(k0t_i, k0t[i], KT, chunks)
                ex_in[i] = (qt_i, k0t_i)

            bias_t = const_pool.tile([1, H], FP16)
            nc.sync.dma_start(bias_t[:], bias[:])
            load_example(0, 8)
            c1t_t = const_pool.tile([128, KT, S], FP16)
            dma_tiled(c1t_t, c1t, KT, 4)
            load_example(1, 4)
            w2_t = const_pool.tile([128, KT, H], FP16)
            dma_tiled(w2_t, w2, KT, 2)
            w01_t = const_pool.tile([128, 2 * KT, H], FP16)
            w01v = w01.rearrange("(a k) h -> a k h", a=2)
            dma_tiled(w01_t[:, KT:2 * KT, :], w01v[1], KT, 2)  # W1 needed first
            dma_tiled(w01_t[:, 0:KT, :], w01v[0], KT, 2)

            ident_h = const_pool.tile([128, 128], FP16)
            make_identity(nc, ident_h[:])
            ones_t = const_pool.tile([1, 128], FP16)
            nc.gpsimd.memset(ones_t[:], 1.0)

            m2_t = const_pool.tile([128, S // 128, H], FP16)

            def emit_m2():
                # M2 = set_ctx @ W2^T + 1*b  [S, H] fp16, once per core
                for si in range(S // 128):
                    for hf in range(2):
                        hsl = slice(hf * 512, (hf + 1) * 512)
                        ps = ps_main.tile([128, 512], F32, tag="ps")
                        for ki in range(KT):
                            nc.tensor.matmul(
                                ps[:],
                                c1t_t[:, ki, si * 128:(si + 1) * 128],
                                w2_t[:, ki, hsl],
                                start=(ki == 0),
                                stop=False,
                            )
                        nc.tensor.matmul(
                            ps[:], ones_t[0:1, :], bias_t[0:1, hsl],
                            start=False, stop=True,
                        )
                        nc.scalar.copy(m2_t[:, si, hsl], ps[:])

            # ---- per-example pipeline: scores run 2 examples ahead ----
            def emit_scores(i):
                if i not in ex_in:
                    load_example(i, 2)
                qt_i, k0t_i = ex_in.pop(i)
                attn_ex_tiles = []
                attn_set_tiles = []
                for li in range(L // 128):
                    lsl = slice(li * 128, (li + 1) * 128)
                    ps_ex = ps_sc.tile([128, 512], F32, tag="psc")
                    for ki in range(KT):
                        nc.tensor.matmul(
                            ps_ex[:, 0:L],
                            qt_i[:, ki, lsl],
                            k0t_i[:, ki, :],
                            start=(ki == 0),
                            stop=(ki == KT - 1),
                        )
                    ps_set = ps_sc.tile([128, 512], F32, tag="psc")
                    for ki in range(KT):
                        nc.tensor.matmul(
                            ps_set[:],
                            qt_i[:, ki, lsl],
                            c1t_t[:, ki, :],
                            start=(ki == 0),
                            stop=(ki == KT - 1),
                        )

                    # softmax over the free axis: p = exp(s - max), r = 1/sum
                    attn_ex = soft_pool.tile([128, L], F32, tag="attn_ex")
                    attn_set = soft_pool.tile([128, S], F32, tag="attn_set")
                    attn_ex_h = soft_pool.tile([128, L], FP16, tag="attn_ex_h")
                    attn_set_h = soft_pool.tile([128, S], FP16, tag="attn_set_h")
                    for ps_s, attn, attn_h in (
                        (ps_ex[:, 0:L], attn_ex, attn_ex_h),
                        (ps_set[:], attn_set, attn_set_h),
                    ):
                        nmx = stats_pool.tile([128, 1], F32, tag="nmx")
                        nc.vector.reduce_max(nmx[:], ps_s, axis=AX, negate=True)
                        psum_r = stats_pool.tile([128, 1], F32, tag="psum")
                        nc.scalar.activation(
                            attn[:], ps_s, ACT.Exp, bias=nmx[:], accum_out=psum_r[:]
                        )
                        rinv = stats_pool.tile([128, 1], F32, tag="rinv")
                        nc.vector.reciprocal(rinv[:], psum_r[:])
                        nc.vector.tensor_scalar_mul(attn_h[:], attn[:], rinv[:])
                        nc.vector.tensor_scalar_mul(attn[:], attn[:], rinv[:])

                    nc.sync.dma_start(exattn[i, lsl, :], attn_ex[:])
                    nc.sync.dma_start(setattn[i, lsl, :], attn_set[:])
                    attn_ex_tiles.append(attn_ex_h)
                    attn_set_tiles.append(attn_set_h)
                return qt_i, k0t_i, attn_ex_tiles, attn_set_tiles

            def emit_rest(i, staged):
                qt_i, k0t_i, attn_ex_tiles, attn_set_tiles = staged

                # M1 = context0 @ W1^T  [L, H] fp16
                m1_t = m1_pool.tile([128, L // 128, H], FP16, tag="m1")
                for si in range(L // 128):
                    for hf in range(2):
                        ps = ps_main.tile([128, 512], F32, tag="ps")
                        for ki in range(KT):
                            nc.tensor.matmul(
                                ps[:],
                                k0t_i[:, ki, si * 128:(si + 1) * 128],
                                w01_t[:, KT + ki, hf * 512:(hf + 1) * 512],
                                start=(ki == 0),
                                stop=(ki == KT - 1),
                            )
                        nc.scalar.copy(m1_t[:, si, hf * 512:(hf + 1) * 512], ps[:])

                # transpose attn -> [s_part, s_tile, l] fp16 (PE transpose)
                at_ex = att_pool.tile([128, L // 128, L], FP16, tag="at_ex")
                for si in range(L // 128):
                    ps_t = ps_tr.tile([128, L], FP16, tag="tr")
                    for li in range(L // 128):
                        nc.tensor.transpose(
                            ps_t[:, li * 128:(li + 1) * 128],
                            attn_ex_tiles[li][:, si * 128:(si + 1) * 128],
                            ident_h[:],
                        )
                    nc.vector.tensor_copy(at_ex[:, si, :], ps_t[:])
                at_set = att_pool.tile([128, S // 128, L], FP16, tag="at_set")
                for si in range(S // 128):
                    ps_t = ps_tr.tile([128, L], FP16, tag="tr")
                    for li in range(L // 128):
                        nc.tensor.transpose(
                            ps_t[:, li * 128:(li + 1) * 128],
                            attn_set_tiles[li][:, si * 128:(si + 1) * 128],
                            ident_h[:],
                        )
                    nc.vector.tensor_copy(at_set[:, si, :], ps_t[:])

                # out = tanh(q @ W0^T + ex_attn @ M1 + set_attn @ (M2 + 1*b))
                for li in range(L // 128):
                    lsl = slice(li * 128, (li + 1) * 128)
                    for hf in range(2):
                        hsl = slice(hf * 512, (hf + 1) * 512)
                        ps = ps_main.tile([128, 512], F32, tag="ps")
                        for ki in range(KT):
                            nc.tensor.matmul(
                                ps[:], qt_i[:, ki, lsl], w01_t[:, ki, hsl],
                                start=(ki == 0), stop=False,
                            )
                        for si in range(L // 128):
                            nc.tensor.matmul(
                                ps[:], at_ex[:, si, lsl], m1_t[:, si, hsl],
                                start=False, stop=False,
                            )
                        for si in range(S // 128):
                            nc.tensor.matmul(
                                ps[:], at_set[:, si, lsl], m2_t[:, si, hsl],
                                start=False, stop=(si == S // 128 - 1),
                            )
                        osb = osb_pool.tile([128, 512], F32, tag="osb")
                        nc.scalar.activation(osb[:], ps[:], ACT.Tanh)
                        nc.sync.dma_start(out[i, lsl, hsl], osb[:])

            staged = {}
            staged[0] = emit_scores(0)
            emit_m2()
            for i in range(NE):
                if i + 1 < NE:
                    staged[i + 1] = emit_scores(i + 1)
                emit_rest(i, staged.pop(i))

    nc.compile()
    return nc


def _get_nc():
    if "nc" not in _CACHE:
        _CACHE["nc"] = _build()
    return _CACHE["nc"]


def kernel_run(output, context0, context1, mask, W_out, b_out, **run_kwargs):
    del mask  # reference discards the masked_fill result; mask is a no-op
    nc = _get_nc()

    # contraction-major fp16 host layouts
    qt = np.ascontiguousarray(output.transpose(0, 2, 1)).astype(np.float16)
    k0t = np.ascontiguousarray(context0.transpose(0, 2, 1)).astype(np.float16)
    c1t = np.ascontiguousarray(context1.transpose(0, 2, 1)).astype(np.float16)
    wt = W_out.T.astype(np.float16)  # [3H, H]
    w01 = np.ascontiguousarray(wt[:2 * H])
    w2 = np.ascontiguousarray(wt[2 * H:])
    b = np.asarray(b_out, dtype=np.float16).reshape(1, H)

    in_maps = []
    for c in range(N_CORES):
        in_maps.append({
            "qt": qt[c * NE:(c + 1) * NE],
            "k0t": k0t[c * NE:(c + 1) * NE],
            "c1t": c1t[c],
            "w01": w01,
            "w2": w2,
            "bias": b,
        })

    res = run_bass_kernel_spmd(nc, in_maps, core_ids=list(range(N_CORES)), **run_kwargs)
    out = np.concatenate([r["out"] for r in res.results], axis=0)
    ex_attn = np.concatenate([r["exattn"] for r in res.results], axis=0)
    set_attn = np.concatenate([r["setattn"] for r in res.results], axis=0)
    return (out, ex_attn, set_attn), res


def kernel(output, context0, context1, mask, W_out, b_out):
    (out, ex_attn, set_attn), _ = kernel_run(
        output, context0, context1, mask, W_out, b_out
    )
    return out, ex_attn, set_attn


# revision 19
# speedup vs baseline: 1.0397x; 1.0127x over previous
"""Trainium2 Bass kernel for nn_Attention_73813307949260.

Reference computation (per example b of 80, L=256, S=512, H=1024):
    ex_attn  = softmax(output @ context0^T)            [L, L]
    set_attn = softmax(output @ set_ctx^T)             [L, S]   set_ctx = context1[b // 10]
    ex_c     = ex_attn @ context0                      [L, H]
    set_c    = set_attn @ set_ctx                      [L, H]
    out      = tanh(concat([output, ex_c, set_c]) @ W_out^T + b_out)
    returns (out, ex_attn, set_attn)                   (mask is unused upstream)

Sharding: data-parallel over the 80-example batch axis, 10 examples per core;
core i's 10 examples all share set_ctx = context1[i]. W_out/b_out replicated.

Kernel algebra (avoids materializing ex_c / set_c):
    out = tanh(output @ W0^T + ex_attn @ M1 + set_attn @ (M2 + 1*b))
    M1 = context0 @ W1^T (per example), M2 = set_ctx @ W2^T (once per core),
    where W_out = [W0 | W1 | W2] split along its 3H input axis. The bias rides
    on M2 because softmax rows sum to 1.

Precision: all matmuls in fp16 (10-bit mantissa) with fp32 PSUM accumulation.
exp() amplifies the *absolute* score error, so bf16 scores (abs err ~0.3 for
std-32 scores) would corrupt the attention weights; fp16 keeps it at ~0.03.
Softmax itself runs in fp32 on the Scalar/Vector engines.
"""

import numpy as np

import concourse.bacc as bacc
import concourse.mybir as mybir
import concourse.tile as tile
from concourse.bass_utils import run_bass_kernel_spmd
from concourse.masks import make_identity

N_CORES = 8
NE = 10          # examples per core
L = 256          # query length
S = 512          # set context length
H = 1024         # hidden
KT = H // 128    # contraction tiles for H
F32 = mybir.dt.float32
FP16 = mybir.dt.float16
AX = mybir.AxisListType.X
ACT = mybir.ActivationFunctionType

_CACHE = {}


def _build():
    nc = bacc.Bacc("TRN2", num_devices=N_CORES)

    # Host passes contraction-major ("transposed") fp16 tensors so every
    # matmul operand loads with the contraction dim on partitions.
    qt = nc.declare_dram_parameter("qt", [NE, H, L], FP16, isOutput=False)
    k0t = nc.declare_dram_parameter("k0t", [NE, H, L], FP16, isOutput=False)
    c1t = nc.declare_dram_parameter("c1t", [H, S], FP16, isOutput=False)
    w01 = nc.declare_dram_parameter("w01", [2 * H, H], FP16, isOutput=False)
    w2 = nc.declare_dram_parameter("w2", [H, H], FP16, isOutput=False)
    bias = nc.declare_dram_parameter("bias", [1, H], FP16, isOutput=False)
    out = nc.declare_dram_parameter("out", [NE, L, H], FP16, isOutput=True)
    exattn = nc.declare_dram_parameter("exattn", [NE, L, L], FP16, isOutput=True)
    setattn = nc.declare_dram_parameter("setattn", [NE, L, S], FP16, isOutput=True)

    def dma_tiled(dst, src, n_kt, chunks):
        """DMA a [n_kt*128, w] DRAM tensor (or [n_kt, 128, w] view) into a
        [128, n_kt, w] tile, split into `chunks` DMAs across HWDGE queues."""
        per = n_kt // chunks
        if len(src.shape) == 2:
            view = src.rearrange("(kt p) w -> p kt w", p=128)
        else:
            view = src.rearrange("kt p w -> p kt w")
        for c in range(chunks):
            nc.sync.dma_start(
                dst[:, c * per:(c + 1) * per, :],
                view[:, c * per:(c + 1) * per, :],
            )

    with tile.TileContext(nc) as tc:
        with (
            tc.tile_pool(name="const", bufs=1) as const_pool,
            tc.tile_pool(name="qk", bufs=4) as qk_pool,
            tc.tile_pool(name="m1", bufs=2) as m1_pool,
            tc.tile_pool(name="att", bufs=2) as att_pool,
            tc.tile_pool(name="soft", bufs=6) as soft_pool,
            tc.tile_pool(name="stats", bufs=12) as stats_pool,
            tc.tile_pool(name="osb", bufs=3) as osb_pool,
            tc.tile_pool(name="ps_main", bufs=6, space="PSUM") as ps_main,
            tc.tile_pool(name="ps_tr", bufs=2, space="PSUM") as ps_tr,
        ):
            # ---- example input loads (ex 0/1 first so the PE starts ASAP) ----
            ex_in = {}

            def load_example(i, chunks):
                qt_i = qk_pool.tile([128, KT, L], FP16, tag="qt")
                k0t_i = qk_pool.tile([128, KT, L], FP16, tag="k0t")
                per = KT // chunks
                qv = qt[i].rearrange("(kt p) w -> p kt w", p=128)
                kv = k0t[i].rearrange("(kt p) w -> p kt w", p=128)
                for c in range(chunks):
                    csl = slice(c * per, (c + 1) * per)
                    nc.sync.dma_start(qt_i[:, csl, :], qv[:, csl, :])
                    nc.sync.dma_start(k0t_i[:, csl, :], kv[:, csl, :])
                ex_in[i] = (qt_i, k0t_i)

            bias_t = const_pool.tile([1, H], FP16)
            nc.sync.dma_start(bias_t[:], bias[:])
            load_example(0, 8)
            c1t_t = const_pool.tile([128, KT, S], FP16)
            dma_tiled(c1t_t, c1t, KT, 4)
            w2_t = const_pool.tile([128, KT, H], FP16)
            dma_tiled(w2_t, w2, KT, 2)
            load_example(1, 4)
            w01_t = const_pool.tile([128, 2 * KT, H], FP16)
            w01v = w01.rearrange("(a k) h -> a k h", a=2)
            dma_tiled(w01_t[:, KT:2 * KT, :], w01v[1], KT, 2)  # W1 needed first
            dma_tiled(w01_t[:, 0:KT, :], w01v[0], KT, 2)

            ident_h = const_pool.tile([128, 128], FP16)
            make_identity(nc, ident_h[:])
            ones_t = const_pool.tile([1, 128], FP16)
            nc.gpsimd.memset(ones_t[:], 1.0)

            m2_t = const_pool.tile([128, S // 128, H], FP16)

            def emit_m2():
                # M2 = set_ctx @ W2^T + 1*b  [S, H] fp16, once per core
                for si in range(S // 128):
                    for hf in range(2):
                        hsl = slice(hf * 512, (hf + 1) * 512)
                        ps = ps_main.tile([128, 512], F32, tag="ps")
                        for ki in range(KT):
                            nc.tensor.matmul(
                                ps[:],
                                c1t_t[:, ki, si * 128:(si + 1) * 128],
                                w2_t[:, ki, hsl],
                                start=(ki == 0),
                                stop=False,
                            )
                        nc.tensor.matmul(
                            ps[:], ones_t[0:1, :], bias_t[0:1, hsl],
                            start=False, stop=True,
                        )
                        nc.scalar.copy(m2_t[:, si, hsl], ps[:])

            # ---- per-example pipeline: scores run 2 examples ahead ----
            def emit_scores(i):
                if i not in ex_in:
                    load_example(i, 2)
                qt_i, k0t_i = ex_in.pop(i)
                attn_ex_tiles = []
                attn_set_tiles = []
                for li in range(L // 128):
                    lsl = slice(li * 128, (li + 1) * 128)
                    ps_ex = ps_main.tile([128, 512], F32, tag="ps")
                    for ki in range(KT):
                        nc.tensor.matmul(
                            ps_ex[:, 0:L],
                            qt_i[:, ki, lsl],
                            k0t_i[:, ki, :],
                            start=(ki == 0),
                            stop=(ki == KT - 1),
                        )
                    ps_set = ps_main.tile([128, 512], F32, tag="ps")
                    for ki in range(KT):
                        nc.tensor.matmul(
                            ps_set[:],
                            qt_i[:, ki, lsl],
                            c1t_t[:, ki, :],
                            start=(ki == 0),
                            stop=(ki == KT - 1),
                        )

                    # softmax over the free axis: p = exp(s - max), r = 1/sum
                    attn_ex = soft_pool.tile([128, L], F32, tag="attn_ex")
                    attn_set = soft_pool.tile([128, S], F32, tag="attn_set")
                    attn_ex_h = soft_pool.tile([128, L], FP16, tag="attn_ex_h")
                    attn_set_h = soft_pool.tile([128, S], FP16, tag="attn_set_h")
                    for ps_s, attn, attn_h in (
                        (ps_ex[:, 0:L], attn_ex, attn_ex_h),
                        (ps_set[:], attn_set, attn_set_h),
                    ):
                        nmx = stats_pool.tile([128, 1], F32, tag="nmx")
                        nc.vector.reduce_max(nmx[:], ps_s, axis=AX, negate=True)
                        psum_r = stats_pool.tile([128, 1], F32, tag="psum")
                        nc.scalar.activation(
                            attn[:], ps_s, ACT.Exp, bias=nmx[:], accum_out=psum_r[:]
                        )
                        rinv = stats_pool.tile([128, 1], F32, tag="rinv")
                        nc.vector.reciprocal(rinv[:], psum_r[:])
                        nc.vector.tensor_scalar_mul(attn_h[:], attn[:], rinv[:])

                    nc.sync.dma_start(exattn[i, lsl, :], attn_ex_h[:])
                    nc.sync.dma_start(setattn[i, lsl, :], attn_set_h[:])
                    attn_ex_tiles.append(attn_ex_h)
                    attn_set_tiles.append(attn_set_h)
                return qt_i, k0t_i, attn_ex_tiles, attn_set_tiles

            def emit_rest(i, staged):
                qt_i, k0t_i, attn_ex_tiles, attn_set_tiles = staged

                # M1 = context0 @ W1^T  [L, H] fp16
                m1_t = m1_pool.tile([128, L // 128, H], FP16, tag="m1")
                for si in range(L // 128):
                    for hf in range(2):
                        ps = ps_main.tile([128, 512], F32, tag="ps")
                        for ki in range(KT):
                            nc.tensor.matmul(
                                ps[:],
                                k0t_i[:, ki, si * 128:(si + 1) * 128],
                                w01_t[:, KT + ki, hf * 512:(hf + 1) * 512],
                                start=(ki == 0),
                                stop=(ki == KT - 1),
                            )
                        nc.scalar.copy(m1_t[:, si, hf * 512:(hf + 1) * 512], ps[:])

                # transpose attn -> [s_part, s_tile, l] fp16 (PE transpose)
                # set first: the final groups consume at_ex earlier than at_set
                at_set = att_pool.tile([128, S // 128, L], FP16, tag="at_set")
                for si in range(S // 128):
                    ps_t = ps_tr.tile([128, L], FP16, tag="tr")
                    for li in range(L // 128):
                        nc.tensor.transpose(
                            ps_t[:, li * 128:(li + 1) * 128],
                            attn_set_tiles[li][:, si * 128:(si + 1) * 128],
                            ident_h[:],
                        )
                    nc.vector.tensor_copy(at_set[:, si, :], ps_t[:])
                at_ex = att_pool.tile([128, L // 128, L], FP16, tag="at_ex")
                for si in range(L // 128):
                    ps_t = ps_tr.tile([128, L], FP16, tag="tr")
                    for li in range(L // 128):
                        nc.tensor.transpose(
                            ps_t[:, li * 128:(li + 1) * 128],
                            attn_ex_tiles[li][:, si * 128:(si + 1) * 128],
                            ident_h[:],
                        )
                    nc.vector.tensor_copy(at_ex[:, si, :], ps_t[:])

                # out = tanh(q @ W0^T + ex_attn @ M1 + set_attn @ (M2 + 1*b))
                for li in range(L // 128):
                    lsl = slice(li * 128, (li + 1) * 128)
                    for hf in range(2):
                        hsl = slice(hf * 512, (hf + 1) * 512)
                        ps = ps_main.tile([128, 512], F32, tag="ps")
                        for ki in range(KT):
                            nc.tensor.matmul(
                                ps[:], qt_i[:, ki, lsl], w01_t[:, ki, hsl],
                                start=(ki == 0), stop=False,
                            )
                        for si in range(L // 128):
                            nc.tensor.matmul(
                                ps[:], at_ex[:, si, lsl], m1_t[:, si, hsl],
                                start=False, stop=False,
                            )
                        for si in range(S // 128):
                            nc.tensor.matmul(
                                ps[:], at_set[:, si, lsl], m2_t[:, si, hsl],
                                start=False, stop=(si == S // 128 - 1),
                            )
                        osb = osb_pool.tile([128, 512], FP16, tag="osb")
                        nc.scalar.activation(osb[:], ps[:], ACT.Tanh)
                        nc.sync.dma_start(out[i, lsl, hsl], osb[:])

            staged = {}
            staged[0] = emit_scores(0)
            emit_m2()
            for i in range(NE):
                if i + 1 < NE:
                    staged[i + 1] = emit_scores(i + 1)
                emit_rest(i, staged.pop(i))

    nc.compile()
    return nc


def _get_nc():
    if "nc" not in _CACHE:
        _CACHE["nc"] = _build()
    return _CACHE["nc"]


def kernel_run(output, context0, context1, mask, W_out, b_out, **run_kwargs):
    del mask  # reference discards the masked_fill result; mask is a no-op
    nc = _get_nc()

    output = np.asarray(output)
    context0 = np.asarray(context0)
    context1 = np.asarray(context1)
    W_out = np.asarray(W_out)
    b_out = np.asarray(b_out)

    # contraction-major fp16 host layouts
    qt = np.ascontiguousarray(output.transpose(0, 2, 1)).astype(np.float16)
    k0t = np.ascontiguousarray(context0.transpose(0, 2, 1)).astype(np.float16)
    c1t = np.ascontiguousarray(context1.transpose(0, 2, 1)).astype(np.float16)
    wt = W_out.T.astype(np.float16)  # [3H, H]
    w01 = np.ascontiguousarray(wt[:2 * H])
    w2 = np.ascontiguousarray(wt[2 * H:])
    b = np.asarray(b_out, dtype=np.float16).reshape(1, H)

    in_maps = []
    for c in range(N_CORES):
        in_maps.append({
            "qt": qt[c * NE:(c + 1) * NE],
            "k0t": k0t[c * NE:(c + 1) * NE],
            "c1t": c1t[c],
            "w01": w01,
            "w2": w2,
            "bias": b,
        })

    res = run_bass_kernel_spmd(nc, in_maps, core_ids=list(range(N_CORES)), **run_kwargs)
    out = np.concatenate([r["out"] for r in res.results], axis=0).astype(np.float32)
    ex_attn = np.concatenate(
        [r["exattn"] for r in res.results], axis=0).astype(np.float32)
    set_attn = np.concatenate(
        [r["setattn"] for r in res.results], axis=0).astype(np.float32)
    return (out, ex_attn, set_attn), res


def kernel(output, context0, context1, mask, W_out, b_out):
    (out, ex_attn, set_attn), _ = kernel_run(
        output, context0, context1, mask, W_out, b_out
    )
    return out, ex_attn, set_attn
